# revision 29
# baseline (speedup 1.0000x reference)
"""Performer self-attention on 8 Trainium2 NeuronCores (linear-attention form).

Sharding: 2 heads per core (head-parallel). Each core computes Q/K/V
projections for its 128-feature slice over all 4096 tokens, then uses the
associativity of the attention product:

    context = (Q' @ K'^T) @ V  ==  Q' @ (K'^T @ V)

(Q' row-normalized positive features; identical math to the reference up to
fp summation order), then a partial output projection over its 128 features.
Host sums the 8 partials + bo.

Program order per core: K+V projections for all 8 token tiles first (KV per
batch finishes as early as possible), then per token tile: Q projection ->
normalization -> C^T = KV-contract -> partial out-projection -> store. The
exposed tail is just the last tile's chain.
"""

import os
import sys

import numpy as np

for _p in ("/opt/trn_rl_repo", "/root/.axon_site/_ro/trn_rl_repo"):
    if _p not in sys.path and os.path.isdir(_p):
        sys.path.append(_p)

B, L, D = 2, 2048, 1024
H = 16
HD = D // H            # 64
NCORES = 8
FPC = (H // NCORES) * HD   # features per core = 128
T = B * L              # 4096
P = 128
KO = D // P            # 8 contraction chunks
NT = 512               # projection token tile
NTT = T // NT          # 8
LT = 512               # context l-tile (== NT)
MCH = L // P           # 16 m-chunks per batch
EPS = 1e-6

_CACHE = {}
LAST_RESULTS = None


def _build_program():
    import concourse.tile as tile
    from concourse import bacc, mybir
    from concourse.masks import make_identity

    f32 = mybir.dt.float32
    f32r = mybir.dt.float32r
    bf16 = mybir.dt.bfloat16
    Exp = mybir.ActivationFunctionType.Exp
    mult = mybir.AluOpType.mult
    add = mybir.AluOpType.add

    nc = bacc.Bacc()

    hsT = nc.declare_dram_parameter("hsT", [P, KO, T], bf16, isOutput=False)
    wq = nc.declare_dram_parameter("wq", [P, KO, FPC], bf16, isOutput=False)
    wk = nc.declare_dram_parameter("wk", [P, KO, FPC], bf16, isOutput=False)
    wv = nc.declare_dram_parameter("wv", [P, KO, FPC], bf16, isOutput=False)
    wo = nc.declare_dram_parameter("wo", [FPC, D], f32r, isOutput=False)
    bq = nc.declare_dram_parameter("bq", [FPC, 1], f32, isOutput=False)
    bk = nc.declare_dram_parameter("bk", [FPC, 1], f32, isOutput=False)
    bv = nc.declare_dram_parameter("bv", [FPC, 1], f32, isOutput=False)
    dmask = nc.declare_dram_parameter("dmask", [P, 2], f32r, isOutput=False)
    hmask = nc.declare_dram_parameter("hmask", [2, P], f32r, isOutput=False)
    out = nc.declare_dram_parameter("out", [T, D], bf16, isOutput=True)

    with tile.TileContext(nc) as tc:
        with tc.tile_pool(name="consts", bufs=1) as consts, \
             tc.tile_pool(name="persist", bufs=1) as persist, \
             tc.tile_pool(name="work", bufs=1) as work, \
             tc.tile_pool(name="pp", bufs=2, space="PSUM") as pp, \
             tc.tile_pool(name="pop", bufs=2, space="PSUM") as pop, \
             tc.tile_pool(name="pstat", bufs=2, space="PSUM") as pstat, \
             tc.tile_pool(name="pkvct", bufs=2, space="PSUM") as pkvct:

            # ---- resident hidden-state tiles; tile 0 + weights first so
            # the first projection can start as early as possible ----
            hts = []
            for i in range(NTT):
                ht = work.tile([P, KO, NT], bf16, tag=f"ht{i}", name=f"ht_{i}")
                hts.append(ht)
            nc.sync.dma_start(hts[0][:], hsT[:, :, 0:NT])

            # ---- constants ----
            wk_sb = consts.tile([P, KO, FPC], bf16, name="wk_sb")
            wv_sb = consts.tile([P, KO, FPC], bf16, name="wv_sb")
            wq_sb = consts.tile([P, KO, FPC], bf16, name="wq_sb")
            wo_sb = consts.tile([FPC, D], f32r, name="wo_sb")
            bq_sb = consts.tile([FPC, 1], f32, name="bq_sb")
            bk_sb = consts.tile([FPC, 1], f32, name="bk_sb")
            bv_sb = consts.tile([FPC, 1], f32, name="bv_sb")
            dmask_sb = consts.tile([P, 2], f32r, name="dmask_sb")
            hmask_sb = consts.tile([2, P], f32r, name="hmask_sb")
            ident_sb = consts.tile([P, P], f32, name="ident_sb")
            identb_sb = consts.tile([P, P], bf16, name="identb_sb")
            nc.sync.dma_start(wk_sb[:], wk[:])
            nc.sync.dma_start(wv_sb[:], wv[:])
            nc.sync.dma_start(wq_sb[:], wq[:])
            nc.sync.dma_start(dmask_sb[:], dmask[:])
            nc.sync.dma_start(hmask_sb[:], hmask[:])
            nc.sync.dma_start(bk_sb[:], bk[:])
            nc.sync.dma_start(bv_sb[:], bv[:])
            nc.sync.dma_start(bq_sb[:], bq[:])
            make_identity(nc, ident_sb[:])
            nc.vector.tensor_copy(identb_sb[:], ident_sb[:])
            for i in range(1, NTT):
                nc.sync.dma_start(hts[i][:], hsT[:, :, i * NT:(i + 1) * NT])
            nc.sync.dma_start(wo_sb[:], wo[:])

            # ---- persistent activations ----
            # K'/V in natural [token, feat] layout, one contiguous tile per
            # 128-token chunk (DMA-transpose needs a contiguous destination)
            k_nat = [persist.tile([P, FPC], bf16, name=f"k_nat_{ch}")
                     for ch in range(T // P)]
            v_nat = [persist.tile([P, FPC], bf16, name=f"v_nat_{ch}")
                     for ch in range(T // P)]

            def stat_scale(pt, kind, i):
                """exp + per-token scale, returns (e, bc) for the multiply."""
                bias = bq_sb if kind == "q" else bk_sb
                e = work.tile([P, NT], f32r, tag="e", bufs=3,
                              name=f"e_{kind}_{i}")
                nc.scalar.activation(e[:], pt[:], Exp, bias=bias[:])
                den = pstat.tile([2, NT], f32, tag="ps", name=f"den_{kind}_{i}")
                nc.tensor.matmul(den[:], dmask_sb[:], e[:],
                                 start=True, stop=True)
                rec_raw = work.tile([2, NT], f32, tag="rec_raw", bufs=4,
                                    name=f"rec_raw_{kind}_{i}")
                nc.vector.reciprocal_approx_fast(out=rec_raw[:], in_=den[:])
                rec = work.tile([2, NT], f32r, tag="rec", bufs=4,
                                name=f"rec_{kind}_{i}")
                nc.scalar.copy(out=rec[:], in_=rec_raw[:])
                bc = pstat.tile([P, NT], f32, tag="ps", name=f"bc_{kind}_{i}")
                nc.tensor.matmul(bc[:], hmask_sb[:], rec[:],
                                 start=True, stop=True)
                return e, bc

            def proj(wsb, i, name):
                pt = pp.tile([P, NT], f32, tag="pp", name=name)
                for ko in range(KO):
                    nc.tensor.matmul(pt[:], wsb[:, ko, :], hts[i][:, ko, :],
                                     start=(ko == 0), stop=(ko == KO - 1))
                return pt

            def pass_a(i):
                """K and V projections + normalization + transpose to natural."""
                # K
                pt = proj(wk_sb, i, f"pt_k_{i}")
                e, bc = stat_scale(pt, "k", i)
                ktb = work.tile([P, NT], bf16, tag="ktb", bufs=2,
                                name=f"ktb_{i}")
                with nc.allow_low_precision(reason="K' is consumed in bf16"):
                    nc.vector.tensor_tensor(out=ktb[:], in0=e[:], in1=bc[:],
                                            op=mult)
                for j in range(NT // P):
                    tp = pop.tile([P, P], bf16, tag="pop", name=f"tpk_{i}_{j}")
                    nc.tensor.transpose(
                        tp[:], ktb[:, j * P:(j + 1) * P], identb_sb[:])
                    nc.vector.tensor_copy(out=k_nat[i * 4 + j][:], in_=tp[:])
                # V
                pt = proj(wv_sb, i, f"pt_v_{i}")
                vtb = work.tile([P, NT], bf16, tag="vtb", bufs=2,
                                name=f"vtb_{i}")
                nc.scalar.add(out=vtb[:], in_=pt[:], add=bv_sb[:])
                for j in range(NT // P):
                    tp = pop.tile([P, P], bf16, tag="pop", name=f"tpv_{i}_{j}")
                    nc.tensor.transpose(
                        tp[:], vtb[:, j * P:(j + 1) * P], identb_sb[:])
                    nc.vector.tensor_copy(out=v_nat[i * 4 + j][:], in_=tp[:])

            def make_kv(b):
                """KV[j,d] per head for one batch (K=128 chunks over its L)."""
                kvs = work.tile([P, HD], f32r, tag="kvs", bufs=2,
                                name=f"kv_sb_{b}")
                for h in range(2):
                    kvp = pkvct.tile([HD, HD], f32, tag="kvct",
                                     name=f"kvp_{b}_{h}")
                    for m in range(MCH):
                        ch = b * MCH + m
                        nc.tensor.matmul(
                            kvp[:],
                            k_nat[ch][:, h * HD:(h + 1) * HD],
                            v_nat[ch][:, h * HD:(h + 1) * HD],
                            start=(m == 0), stop=(m == MCH - 1))
                    nc.vector.tensor_copy(out=kvs[h * HD:(h + 1) * HD, :],
                                          in_=kvp[:])
                return kvs

            def pass_b(i, kvs):
                """Q projection + normalization + C^T + partial out-proj."""
                ls = i * NT
                pt = proj(wq_sb, i, f"pt_q_{i}")
                e, bc = stat_scale(pt, "q", i)
                qt = work.tile([P, NT], f32r, tag="qt", bufs=2, name=f"qt_{i}")
                nc.vector.tensor_tensor(out=qt[:], in0=e[:], in1=bc[:],
                                        op=mult)
                ct_sb = work.tile([P, LT], f32r, tag="ct_sb", bufs=2,
                                  name=f"ct_sb_{i}")
                for h in range(2):
                    # head A -> PE tile T0, head B -> T8 (rows 64-127)
                    cp = pkvct.tile([HD, LT], f32, tag="kvct",
                                    name=f"cp_{i}_{h}")
                    nc.tensor.matmul(
                        cp[:],
                        kvs[h * HD:(h + 1) * HD, :],
                        qt[h * HD:(h + 1) * HD, :],
                        start=True, stop=True)
                    if h == 0:
                        nc.vector.tensor_copy(
                            out=ct_sb[h * HD:(h + 1) * HD, :], in_=cp[:])
                    else:
                        nc.scalar.copy(
                            out=ct_sb[h * HD:(h + 1) * HD, :], in_=cp[:])
                # partial output projection, staged so the store is one DMA
                ob = work.tile([P, LT // P, D], bf16, tag="ob", bufs=2,
                               name=f"ob_{i}")
                for t in range(LT // P):
                    for n in range(D // NT):
                        op = pop.tile([P, NT], f32, tag="pop",
                                      name=f"op_{i}_{t}_{n}")
                        nc.tensor.matmul(
                            op[:],
                            ct_sb[:, t * P:(t + 1) * P],
                            wo_sb[:, n * NT:(n + 1) * NT],
                            start=True, stop=True)
                        if (t * 2 + n) % 2 == 0:
                            nc.vector.tensor_copy(
                                out=ob[:, t, n * NT:(n + 1) * NT], in_=op[:])
                        else:
                            nc.scalar.copy(
                                out=ob[:, t, n * NT:(n + 1) * NT], in_=op[:])
                nc.sync.dma_start(
                    out[ls:ls + LT, :].rearrange("(t p) d -> p t d", p=P),
                    ob[:])

            # batch 0 K/V, then its KV; batch-1 pass A interleaved with
            # batch-0 attention; finally batch-1 attention.
            for i in range(4):
                pass_a(i)
            kv0 = make_kv(0)
            for i in range(4):
                pass_a(4 + i)
                pass_b(i, kv0)
            kv1 = make_kv(1)
            for i in range(4):
                pass_b(4 + i, kv1)

    nc.compile()
    return nc


def _get_program():
    if "nc" not in _CACHE:
        _CACHE["nc"] = _build_program()
    return _CACHE["nc"]


def kernel(hidden_states, Wq, bq, Wk, bk, Wv, bv, Wo, bo):
    global LAST_RESULTS
    import ml_dtypes
    from concourse.bass_utils import run_bass_kernel_spmd

    bf = ml_dtypes.bfloat16
    hidden_states = np.ascontiguousarray(np.asarray(hidden_states, dtype=np.float32))
    Wq = np.asarray(Wq, dtype=np.float32)
    Wk = np.asarray(Wk, dtype=np.float32)
    Wv = np.asarray(Wv, dtype=np.float32)
    Wo = np.asarray(Wo, dtype=np.float32)
    bq = np.asarray(bq, dtype=np.float32)
    bk = np.asarray(bk, dtype=np.float32)
    bv = np.asarray(bv, dtype=np.float32)
    bo = np.asarray(bo, dtype=np.float32)
    assert hidden_states.shape == (B, L, D)

    nc = _get_program()

    hs_flat = hidden_states.reshape(T, D)
    # hsT[p, ko, t] = hs_flat[t, ko*128+p]
    hsT = np.ascontiguousarray(
        hs_flat.T.reshape(KO, P, T).transpose(1, 0, 2)).astype(bf)

    pidx = np.arange(P)
    dmask = (pidx[:, None] // HD == np.arange(2)[None, :]).astype(np.float32)
    hmask = np.ascontiguousarray(dmask.T)

    def wslice(W, c):
        # [128, 8, 128]: w[p, ko, m] = W[ko*128+p, c*128+m]
        return np.ascontiguousarray(
            W[:, c * FPC:(c + 1) * FPC].reshape(KO, P, FPC)
            .transpose(1, 0, 2)).astype(bf)

    in_maps = []
    for c in range(NCORES):
        sl = slice(c * FPC, (c + 1) * FPC)
        in_maps.append({
            "hsT": hsT,
            "wq": wslice(Wq, c),
            "wk": wslice(Wk, c),
            "wv": wslice(Wv, c),
            "wo": np.ascontiguousarray(Wo[sl, :]),
            "bq": np.ascontiguousarray(bq[sl, None]),
            "bk": np.ascontiguousarray(bk[sl, None]),
            "bv": np.ascontiguousarray(bv[sl, None]),
            "dmask": dmask,
            "hmask": hmask,
        })

    trace = bool(os.environ.get("BASS_TRACE"))
    if trace:
        try:
            from antenv.axon_hooks import get_axon_ntff_profile_hook  # noqa: F401
        except ImportError:
            trace = False
    res = run_bass_kernel_spmd(nc, in_maps, core_ids=list(range(NCORES)),
                               trace=trace)
    LAST_RESULTS = res

    acc = np.zeros((T, D), dtype=np.float64)
    for c in range(NCORES):
        acc += res.results[c]["out"].astype(np.float64)
    acc += bo[None, :]
    return acc.astype(np.float32).reshape(B, L, D)
